# revision 3
# baseline (speedup 1.0000x reference)
"""Trainium2 Bass kernel for nn_ConvMultiHeadAttention.

Data-parallel over batch B=8 across 8 NeuronCores (no collectives).
Per core (one batch element):

  1. Host folds the 1x1 proj into the 3x3 value conv (G_h = Wp_h @ Wv_h) and
     pre-builds a padded, column-shifted, bf16 staging tensor tf1 so the
     device does no staging copies: partitions 0:64 hold each frame padded
     (A), partitions 64:128 hold the same frame shifted one column (B).
     Taps (ky,0)+(ky,1) pair into single K=128 matmuls; the three kx=2 taps
     run as K=64 matmuls on one array half each (row-group concurrent).
  2. Conv loop is frame-outer: per (frame, head-pair) the full-frame
     [128, 1024] bf16 result is evicted from PSUM (DVE cast) and scattered
     with ONE 256KB SBUF->SBUF DMA into zT[(frame,head), (c,pos)] layout,
     alternating the sync/scalar HWDGE queues so DMAs pipeline.
  3. q/k projections + masked softmax (tiny) overlap the conv.
  4. Attention mix: 128 matmuls contract (frame, head) at once over the
     full-frame zT; outputs batch 4 slices per store DMA.

Softmax rows sum to 1, so conv bias (Wp@bv) and proj bias bp reduce to a
per-channel constant added on the host.
"""

import os
import numpy as np

import concourse.bass as bass
import concourse.bacc as bacc
import concourse.tile as tile
import concourse.mybir as mybir
from concourse.bass_utils import run_bass_kernel_spmd

NH, DQK, DV = 8, 256, 64
B, TI, TO, H, W = 8, 16, 16, 32, 32
HW = H * W           # 1024
N_CORES = 8

F32 = mybir.dt.float32
BF16 = mybir.dt.bfloat16
I32 = mybir.dt.int32

_GRAPH = None
LAST_RESULTS = None


def _build_graph():
    from contextlib import ExitStack

    nc = bacc.Bacc("TRN2", target_bir_lowering=False, debug=False,
                   num_devices=N_CORES)

    tf1_ap = nc.dram_tensor("tf1", [128, TI * 34 * 34], BF16, kind="ExternalInput").ap()
    wc_ap = nc.dram_tensor("wc", [128, 3072], BF16, kind="ExternalInput").ap()
    wqk_ap = nc.dram_tensor("wqk", [128, 8192], BF16, kind="ExternalInput").ap()
    q_ap = nc.dram_tensor("q", [TO, DQK], F32, kind="ExternalInput").ap()
    k_ap = nc.dram_tensor("k", [TI, DQK], F32, kind="ExternalInput").ap()
    m_ap = nc.dram_tensor("mask", [TO, TI], I32, kind="ExternalInput").ap()
    bqk_ap = nc.dram_tensor("bqk", [128, 32], F32, kind="ExternalInput").ap()
    id_ap = nc.dram_tensor("ident", [16, 16], F32, kind="ExternalInput").ap()
    out_ap = nc.dram_tensor("out", [TO, DV * HW], F32, kind="ExternalOutput").ap()

    AF = mybir.ActivationFunctionType
    OP = mybir.AluOpType

    with tile.TileContext(nc) as tc, ExitStack() as ctx:
        cps = ctx.enter_context(tc.tile_pool(name="cps", bufs=4, space="PSUM"))
        mps = ctx.enter_context(tc.tile_pool(name="mps", bufs=3, space="PSUM"))
        sps = ctx.enter_context(tc.tile_pool(name="sps", bufs=1, space="PSUM"))
        zfrp = ctx.enter_context(tc.tile_pool(name="zfrp", bufs=3))
        wqkp = ctx.enter_context(tc.tile_pool(name="wqkp", bufs=2))
        attp = ctx.enter_context(tc.tile_pool(name="attp", bufs=2))

        def static(name, shape, dtype):
            return nc.alloc_sbuf_tensor(name, list(shape), dtype).ap()

        tf1 = static("tf1_sb", [128, TI * 34 * 34], BF16)
        zT = static("zT", [128, 64 * 1024], BF16)
        wc = static("wc_sb", [128, 3072], BF16)
        qk = static("qk_sb", [16, 512], F32)
        qkT = static("qkT", [128, 64], BF16)
        pqT = static("pqT", [128, 256], BF16)
        pkT = static("pkT", [128, 256], BF16)
        wflat = static("wflat", [128, 16], BF16)
        id_sb = static("id_sb", [16, 16], F32)
        bqk = static("bqk_sb", [128, 32], F32)
        mi = static("mi_sb", [16, 16], I32)
        mb = static("mb", [16, 16], F32)
        mbig = static("mbig", [16, 128], F32)
        s1 = static("s1", [16, 128], F32)
        s2 = static("s2", [16, 128], F32)
        s3 = static("s3", [16, 128], F32)
        s4 = static("s4", [16, 128], F32)
        rmax = static("rmax", [16, 8], F32)
        rsum = static("rsum", [16, 8], F32)
        rinv = static("rinv", [16, 8], F32)

        t1v = tf1[:].rearrange("p (f r c) -> p f r c", r=34, c=34)

        # ---------- input / constant loads ----------
        nc.scalar.dma_start(wc[:], wc_ap[:, :])
        nc.scalar.dma_start(tf1[:, 0:9248], tf1_ap[:, 0:9248])
        nc.scalar.dma_start(tf1[:, 9248:18496], tf1_ap[:, 9248:18496])
        nc.sync.dma_start(qk[:, 0:256], q_ap[:, :])
        nc.sync.dma_start(qk[:, 256:512], k_ap[:, :])
        nc.sync.dma_start(mi[:], m_ap[:, :])
        nc.sync.dma_start(id_sb[:], id_ap[:, :])
        nc.sync.dma_start(bqk[:], bqk_ap[:, :])

        # ---------- scores / softmax phase ----------
        # qT / kT tiles via PE transpose: qkT cols [q-t0 | q-t1 | k-t0 | k-t1]
        for j in range(4):
            half, t = j // 2, j % 2
            ps = sps.tile([128, 16], F32, name="tps", tag="sps")
            nc.tensor.transpose(
                ps[:], qk[0:16, half * 256 + t * 128: half * 256 + (t + 1) * 128],
                id_sb[:])
            nc.vector.tensor_copy(qkT[:, j * 16:(j + 1) * 16], ps[:])

        # pqT / pkT: per m-tile of 128 (h,d)-rows, contract d' over 2 K-tiles.
        # wqk col block b = src*32 + m*2 + t; loaded in groups of 8 blocks.
        for g in range(8):
            wt = wqkp.tile([128, 1024], BF16, name="wt", tag="wt")
            eng = nc.sync if g % 2 == 0 else nc.scalar
            eng.dma_start(wt[:], wqk_ap[:, g * 1024:(g + 1) * 1024])
            for mloc in range(4):
                src = g // 4
                m = (g % 4) * 4 + mloc
                dst = pqT if src == 0 else pkT
                ps = sps.tile([128, 16], F32, name="pps", tag="sps")
                for t in range(2):
                    nc.tensor.matmul(
                        ps[:], wt[:, (mloc * 2 + t) * 128:(mloc * 2 + t + 1) * 128],
                        qkT[:, (src * 2 + t) * 16:(src * 2 + t + 1) * 16],
                        start=(t == 0), stop=(t == 1))
                nc.vector.tensor_scalar_add(
                    dst[:, m * 16:(m + 1) * 16], ps[:],
                    bqk[:, src * 16 + m: src * 16 + m + 1])

        # scores[o, (h,i)]: per head contract over d (2 m-tiles)
        sc = sps.tile([16, 128], F32, name="sc", tag="sps")
        for h in range(8):
            for t in range(2):
                sl = slice((2 * h + t) * 16, (2 * h + t + 1) * 16)
                nc.tensor.matmul(sc[:, h * 16:(h + 1) * 16], pqT[:, sl], pkT[:, sl],
                                 start=(t == 0), stop=(t == 1))

        # masked softmax over i within each head block
        nc.scalar.activation(s1[:], sc[:], AF.Copy, scale=1.0 / 16.0)
        nc.vector.tensor_copy(mb[:], mi[:])
        nc.vector.tensor_scalar(mb[:], mb[:], 1.0e10, -1.0e10, OP.mult, OP.add)
        for h in range(8):
            nc.vector.tensor_copy(mbig[:, h * 16:(h + 1) * 16], mb[:])
        nc.vector.tensor_tensor(s2[:], s1[:], mbig[:], op=OP.add)
        nc.vector.reduce_max(rmax[:], s2[:].rearrange("p (h i) -> p h i", i=16),
                             axis=mybir.AxisListType.X)
        for h in range(8):
            nc.vector.tensor_scalar(s3[:, h * 16:(h + 1) * 16],
                                    s2[:, h * 16:(h + 1) * 16],
                                    rmax[:, h:h + 1], None, OP.subtract)
        nc.scalar.activation(s4[:], s3[:], AF.Exp)
        nc.vector.reduce_sum(rsum[:], s4[:].rearrange("p (h i) -> p h i", i=16),
                             axis=mybir.AxisListType.X)
        nc.vector.reciprocal(rinv[:], rsum[:])
        # write normalized weights interleaved: s3 free index = i*8 + h, so the
        # transpose below yields wflat partitions p = i*8 + h (zT layout).
        for h in range(8):
            nc.vector.tensor_scalar(s3[:, h::8],
                                    s4[:, h * 16:(h + 1) * 16],
                                    rinv[:, h:h + 1], None, OP.mult)
        wt_ps = sps.tile([128, 16], F32, name="wt_ps", tag="sps")
        nc.tensor.transpose(wt_ps[:], s3[:], id_sb[:])
        nc.vector.tensor_copy(wflat[:], wt_ps[:])

        # ---------- conv, frame-outer; scatter per (frame, head-pair) ----------
        zTv = zT[:].rearrange("p (c n) -> p c n", n=1024)

        for fr in range(16):
            for hp in range(4):
                zfr = zfrp.tile([128, 1024], BF16, name="zfr", tag="zfr")
                for c16 in range(2):
                    y0 = 16 * c16
                    ps = cps.tile([128, 16, 32], F32, name="cpst", tag="cpst")
                    for j in range(6):
                        lhsT = wc[:, (hp * 6 + j) * 128:(hp * 6 + j + 1) * 128]
                        if j < 3:
                            rhs = t1v[:, fr, y0 + j: y0 + j + 16, 0:32]
                        elif j == 3:
                            rhs = t1v[:, fr, y0 + 0: y0 + 16, 2:34]
                        elif j == 4:
                            rhs = t1v[:, fr, y0 + 1: y0 + 17, 1:33]
                        else:
                            rhs = t1v[:, fr, y0 + 2: y0 + 18, 2:34]
                        nc.tensor.matmul(ps[:], lhsT, rhs,
                                         start=(j == 0), stop=(j == 5))
                    nc.vector.tensor_copy(zfr[:, c16 * 512:(c16 + 1) * 512], ps[:])
                # scatter: zT partitions 8fr+2hp+hl; free (c, pos)
                dst = zTv[8 * fr + 2 * hp: 8 * fr + 2 * hp + 2]
                eng = nc.sync if (fr * 4 + hp) % 2 == 0 else nc.scalar
                eng.dma_start(dst, zfr[:])

        # ---------- attention mix + batched stores ----------
        for sb in range(32):
            att = attp.tile([16, 2048], F32, name="attt", tag="attt")
            for k4 in range(4):
                s = sb * 4 + k4
                mp = mps.tile([16, 512], F32, name="mpst", tag="mpst")
                nc.tensor.matmul(mp[:], wflat[:], zT[:, s * 512:(s + 1) * 512],
                                 start=True, stop=True)
                if k4 % 2 == 0:
                    nc.scalar.copy(att[:, k4 * 512:(k4 + 1) * 512], mp[:])
                else:
                    nc.vector.tensor_copy(att[:, k4 * 512:(k4 + 1) * 512], mp[:])
            eng = nc.sync if sb % 2 == 0 else nc.scalar
            eng.dma_start(out_ap[:, sb * 2048:(sb + 1) * 2048], att[:])

    nc.compile()
    return nc


def _host_consts(Wq, bq, Wk, bk, Wv, bv, Wp, bp):
    import ml_dtypes

    Wq = np.asarray(Wq, np.float32)
    Wk = np.asarray(Wk, np.float32)
    Wv = np.asarray(Wv, np.float32)
    Wp = np.asarray(Wp, np.float32)
    bq = np.asarray(bq, np.float32)
    bk = np.asarray(bk, np.float32)
    bv = np.asarray(bv, np.float32)
    bp = np.asarray(bp, np.float32)

    # fold 1x1 proj into the 3x3 conv
    Wv5 = Wv.reshape(NH, DV, DV, 3, 3)
    Wp3 = Wp.reshape(DV, NH, DV)
    G = np.einsum('ohm,hmiyx->hoiyx', Wp3, Wv5).reshape(NH * DV, DV, 3, 3)

    WC = np.zeros((128, 4, 6, 128), np.float32)
    for hp in range(4):
        oc = np.arange(128) + hp * 128
        for ky in range(3):
            WC[0:64, hp, ky, :] = G[oc, :, ky, 0].T
            WC[64:128, hp, ky, :] = G[oc, :, ky, 1].T
        WC[0:64, hp, 3, :] = G[oc, :, 0, 2].T
        WC[64:128, hp, 4, :] = G[oc, :, 1, 2].T
        WC[0:64, hp, 5, :] = G[oc, :, 2, 2].T
    wc = np.ascontiguousarray(WC.reshape(128, 3072)).astype(ml_dtypes.bfloat16)

    wqk = np.zeros((128, 8192), np.float32)
    for i, Wmat in enumerate([Wq, Wk]):
        for m in range(16):
            for t in range(2):
                b = i * 32 + m * 2 + t
                wqk[:, b * 128:(b + 1) * 128] = Wmat[t * 128:(t + 1) * 128,
                                                     m * 128:(m + 1) * 128]
    wqk = np.ascontiguousarray(wqk).astype(ml_dtypes.bfloat16)

    bqk = np.zeros((128, 32), np.float32)
    bqk[:, 0:16] = bq.reshape(16, 128).T
    bqk[:, 16:32] = bk.reshape(16, 128).T

    ident = np.eye(16, dtype=np.float32)
    bias_total = Wp.reshape(DV, NH * DV) @ bv + bp
    return wc, wqk, bqk, ident, bias_total


def _host_tf1(vb):
    """vb: [TI, DV, H, W] f32 -> padded/shifted bf16 staging [128, TI*34*34]."""
    import ml_dtypes
    tf1 = np.zeros((128, TI, 34, 34), np.float32)
    vt = vb.transpose(1, 0, 2, 3)            # [DV, TI, H, W]
    tf1[0:64, :, 1:33, 1:33] = vt
    tf1[64:128, :, 1:33, 0:32] = vt
    return np.ascontiguousarray(tf1.reshape(128, TI * 34 * 34)).astype(
        ml_dtypes.bfloat16)


def _get_graph():
    global _GRAPH
    if _GRAPH is None:
        _GRAPH = _build_graph()
    return _GRAPH


def kernel(v, k, q, prod_mask, Wq, bq, Wk, bk, Wv, bv, Wp, bp):
    global LAST_RESULTS
    nc = _get_graph()
    wc, wqk, bqk, ident, bias_total = _host_consts(Wq, bq, Wk, bk, Wv, bv, Wp, bp)

    v = np.asarray(v, np.float32)
    q = np.ascontiguousarray(np.asarray(q, np.float32))
    k = np.ascontiguousarray(np.asarray(k, np.float32))
    pm = np.ascontiguousarray(np.asarray(prod_mask, np.int32))

    in_maps = []
    for b in range(N_CORES):
        in_maps.append({
            "tf1": _host_tf1(v[b]), "q": q[b], "k": k[b], "mask": pm[b],
            "wc": wc, "wqk": wqk, "bqk": bqk, "ident": ident,
        })

    trace = bool(int(os.environ.get("KERNEL_TRACE", "0")))
    tmpdir = os.environ.get("KERNEL_TRACE_DIR") or None
    res = run_bass_kernel_spmd(nc, in_maps, core_ids=list(range(N_CORES)),
                               trace=trace, tmpdir=tmpdir)
    LAST_RESULTS = res

    out = np.stack([res.results[i]["out"] for i in range(N_CORES)])
    out = out.reshape(B, TO, DV, H, W) + bias_total[None, None, :, None, None]
    return np.ascontiguousarray(out.astype(np.float32))
